# revision 27
# baseline (speedup 1.0000x reference)
"""GRU model Trainium2 Bass kernel (v2: window-in-PSUM recurrence).

Model (V=32000, E=256, H=256, O=32000, B=256, S=512):
  xe = emb[x]                                    # [B,S,E]
  iz/ir/ih = xe @ W{z,r,h}.T + b + bu            # input-side projections
  h_{s+1} = (1-z) h + z tanh(ih + (r h) @ Uh.T)  # 512-step GRU recurrence
  out = h_S @ Wf.T + bf                          # [B,O]

Sharding: data-parallel, batch 256 -> 32 rows per core on 8 cores.

v2 design (vs v1):
  - 4-step windows whose input projections are computed DIRECTLY INTO PSUM
    (three 1-bank tiles per window: [128, 2(m), 4(step), 32(batch)] f32 for
    z / r / h).  The recurrence U-matmuls accumulate start=False onto the
    projections, so there are no eye-seed matmuls and no ScalarE evacuation
    copies.  Separate per-gate tiles keep Tile's (tile-granular) dependency
    tracking from serializing one gate's matmuls behind another's sigmoid.
  - U weights are fp8e4m3 (mixed with bf16 moving operand): FWL loads the
    stationary tile in ~27ns instead of ~53, halving the LDWEIGHTS cost
    that dominates the N=32 recurrence matmuls on hardware (measured
    end-to-end error 8.3e-3 scale-relative vs 4.9e-3 all-bf16).
  - Per step the r-gate is accumulated first and gets its own small
    Sigmoid; z's sigmoid, z*h and h - z*h run in the shadow.  The h-update
    h' = z*tanh + (h - z*h) needs only z*tanh post-tanh, and the next
    step's r-matmuls consume (z*tanh) and (h - z*h) as two separate rhs
    contributions (U h' = U zt + U hk), so the h'-add is off the critical
    path entirely.  Steady-state step = ~1.29us in CoreSim: sigmoid(238) ->
    rh -> 4 Uh-matmuls -> tanh(238) -> z*tanh -> r-matmuls, plus 4x ~100ns
    cross-engine semaphore hops.
  - Next-window projection matmuls are emitted as 128-column quanta and
    drained a few per step so the in-order PE queue never blocks a critical
    recurrence matmul behind a long projection matmul.
  - Wf ([256,32000] bf16 = 16.4MB) is prefetched into SBUF during the
    recurrence; the FC tail (~5us) is compute-only: Wf-stationary matmuls
    (out = [128 vocab, 32 batch]), evacuated by ScalarE/DVE, written to
    DRAM as [128, 250, 32] and transposed on the host (bias also added on
    host -- it enters additively in the final output only).
"""

import sys

if "/opt/trn_rl_repo" not in sys.path:
    sys.path.insert(0, "/opt/trn_rl_repo")

import numpy as np

V, E, H, O = 32000, 256, 256, 32000
B, S = 256, 512
NCORES = 8
BP = B // NCORES          # 32 batch rows per core
WSTEPS = 4                # recurrence steps per window
NW = S // WSTEPS          # 64 windows
WTOK = WSTEPS * BP        # 128 tokens gathered per window
FC_TILES = O // 128       # 250 vocab tiles
FC_GROUP = 16             # vocab tiles per PSUM bank in the FC


def build_kernel(n_windows=NW, interleave=True):
    import concourse.bass as bass
    import concourse.bacc as bacc
    import concourse.mybir as mybir
    from concourse.tile import TileContext
    from concourse import library_config

    f32 = mybir.dt.float32
    bf16 = mybir.dt.bfloat16
    f8 = mybir.dt.float8e4
    i16 = mybir.dt.int16
    AF = mybir.ActivationFunctionType

    n_steps = n_windows * WSTEPS

    nc = bacc.Bacc("TRN2")

    n_idx_cols = n_windows * (WTOK // 16)
    # packed constants, int16-typed raw columns:
    #   [0:1536] wt(bf16) | [1536:2304] ut(fp8e4m3, 2/col) | idx(i16)
    C_WT, C_UT, C_IDX = 0, 1536, 2304
    n_const = C_IDX + n_idx_cols
    const_d = nc.dram_tensor("const2d", [128, n_const], i16, kind="ExternalInput")
    # row-0 constants (bf16): [0:768] gate bias | [768:768+WTOK] ones
    row0_d = nc.dram_tensor("row0", [1, 3 * H + WTOK], bf16, kind="ExternalInput")
    emb_d = nc.dram_tensor("emb_bf", [V, E], bf16, kind="ExternalInput")
    wft_d = nc.dram_tensor("wft", [128, 2, FC_TILES, 128], bf16,
                           kind="ExternalInput")
    out_d = nc.dram_tensor("out", [128, FC_TILES, BP], f32, kind="ExternalOutput")

    with TileContext(nc) as tc:
        with (
            tc.tile_pool(name="const", bufs=1) as cpool,
            tc.tile_pool(name="xe", bufs=3) as xe_pool,
            tc.tile_pool(name="win", bufs=2, space="PSUM") as win_pool,
            tc.tile_pool(name="ew", bufs=2) as ew_pool,
        ):
            # dma_gather is implemented by the Q7 'mlp' library
            nc.gpsimd.load_library(library_config.mlp)

            # ---- constants / weights to SBUF ----
            const_sb = cpool.tile([128, n_const], i16)
            # idx first (gates the first gather); weights ride the Act queue
            nc.sync.dma_start(out=const_sb[:, C_IDX:n_const],
                              in_=const_d[:, C_IDX:n_const])
            nc.scalar.dma_start(out=const_sb[:, 0:C_IDX],
                                in_=const_d[:, 0:C_IDX])
            row0_sb = cpool.tile([1, 3 * H + WTOK], bf16)
            nc.scalar.dma_start(out=row0_sb, in_=row0_d[:, :])
            # FC weights prefetch (completes during the recurrence)
            wf_sb = cpool.tile([128, 2, FC_TILES, 128], bf16)
            tsplit = [0, 63, 126, 188, FC_TILES]
            for q in range(4):
                nc.sync.dma_start(out=wf_sb[:, :, tsplit[q]:tsplit[q + 1]],
                                  in_=wft_d[:, :, tsplit[q]:tsplit[q + 1]])

            wt = const_sb[:, C_WT:C_UT].bitcast(bf16)    # proj lhsT packed
            ut = const_sb[:, C_UT:C_IDX].bitcast(f8)     # rec lhsT packed (fp8)
            idx_sb = const_sb[:, C_IDX:n_const]
            brow = row0_sb[:, 0:3 * H]
            ones = row0_sb[:, 3 * H:3 * H + WTOK]

            # persistent hidden state, double-buffered: [128, 2, 32] bf16
            # (partition = h-dim within k-half, dims = k-half x batch)
            hbuf = [cpool.tile([128, 2, BP], bf16, tag=f"h{i}", name=f"h{i}")
                    for i in range(2)]
            nc.vector.memset(hbuf[0], 0.0)

            win = [None] * n_windows    # per-window PSUM tiles
            xet_w = [None] * n_windows  # per-window gathered embeddings

            def emit_gather(w):
                xet_w[w] = xe_pool.tile([128, 2, WTOK], bf16, tag="xet",
                                        name="xet")
                c0 = w * (WTOK // 16)
                nc.gpsimd.dma_gather(
                    xet_w[w], emb_d[:, :], idx_sb[:, c0:c0 + WTOK // 16],
                    WTOK, WTOK, E, transpose=True,
                )

            def proj_ops(w):
                """Thunks emitting window w's projection matmul quanta.

                win[w] = three 1-bank PSUM tiles [128, m(2), j(WSTEPS),
                b(32)] f32 for z / r / h.  Each bank's first matmul carries
                start=True (the PSUM zero-region is one bank); every later
                write (incl. the recurrence U-matmuls) accumulates.  Matmuls
                are emitted as 128-column quanta so a queued projection
                never blocks a critical recurrence matmul for long.
                """
                win[w] = [win_pool.tile([128, 2, WSTEPS, BP], f32,
                                        tag=f"win{g}", name=f"win{g}")
                          for g in range(3)]
                ops = []
                for g in range(3):
                    for m in range(2):
                        gm = 2 * g + m

                        # 128-col quanta keep head-of-line blocking of the
                        # recurrence matmuls small on the in-order PE queue
                        hs, htok = WSTEPS // 2, WTOK // 2

                        def seed(g=g, m=m, gm=gm, half=0):
                            c = g * H + m * 128
                            nc.tensor.matmul(
                                win[w][g][:, m, half * hs:half * hs + hs],
                                brow[:, c:c + 128],
                                ones[:, half * htok:half * htok + htok],
                                start=(m == 0 and half == 0), stop=False,
                                skip_group_check=True)

                        ops.append(lambda f=seed: f(half=0))
                        ops.append(lambda f=seed: f(half=1))
                        for k in range(2):
                            def mm(g=g, m=m, k=k, gm=gm, half=0):
                                c = k * 3 * H + g * H + m * 128
                                nc.tensor.matmul(
                                    win[w][g][:, m, half * hs:half * hs + hs],
                                    wt[:, c:c + 128],
                                    xet_w[w][:, k, half * htok:half * htok + htok],
                                    start=False, stop=False,
                                    skip_group_check=True)

                            ops.append(lambda f=mm: f(half=0))
                            ops.append(lambda f=mm: f(half=1))
                return ops

            prev_parts = [None]  # (zt2, hkeep) of the previous step

            def emit_step(s, pq):
                w, j = divmod(s, WSTEPS)
                h_in = hbuf[s % 2]
                h_out = hbuf[(s + 1) % 2]
                pw = win[w]

                def umm(g, out_ap, rhs, stop=True):
                    for m in range(2):
                        for k in range(2):
                            c = g * 2 * H + k * H + m * 128
                            nc.tensor.matmul(
                                out_ap(m), ut[:, c:c + 128], rhs[:, k],
                                start=False, stop=(stop and k == 1),
                                skip_group_check=True)

                # r first: its sigmoid gates the critical path.  h' = zt2 +
                # hkeep, and Ur h' = Ur zt2 + Ur hkeep: the hkeep half is
                # ready early (runs under the previous tanh), so only the
                # zt2 half waits -- the h'-add never gates the r-matmuls.
                if prev_parts[0] is None:
                    umm(1, lambda m: pw[1][:, m, j], h_in)
                else:
                    p_zt2, p_hk = prev_parts[0]
                    umm(1, lambda m: pw[1][:, m, j], p_hk, stop=False)
                    umm(1, lambda m: pw[1][:, m, j], p_zt2)
                r_t = ew_pool.tile([128, 2, BP], bf16, tag="rt", name="rt")
                nc.scalar.activation(r_t, pw[1][:, :, j], AF.Sigmoid)
                # z matmuls run on PE while sigma_r executes
                umm(0, lambda m: pw[0][:, m, j], h_in)
                z_t = ew_pool.tile([128, 2, BP], bf16, tag="zt", name="zt")
                nc.scalar.activation(z_t, pw[0][:, :, j], AF.Sigmoid)
                # projections of window w+1 fill PE idle under sigma_r/rh
                for _ in range(5):
                    if pq:
                        pq.pop(0)()
                rh = ew_pool.tile([128, 2, BP], bf16, tag="rh", name="rh")
                nc.vector.tensor_mul(rh, r_t, h_in)
                umm(2, lambda m: pw[2][:, m, j], rh)
                # next window's projections fill the PE idle under tanh/update
                for _ in range(5):
                    if pq:
                        pq.pop(0)()
                # shadow: hz = z*h, hkeep = (1-z)*h -- ready before tanh lands
                hz = ew_pool.tile([128, 2, BP], bf16, tag="hz", name="hz")
                nc.vector.tensor_mul(hz, z_t, h_in)
                hkeep = ew_pool.tile([128, 2, BP], bf16, tag="hk", name="hk")
                nc.vector.tensor_sub(hkeep, h_in, hz)
                ht = ew_pool.tile([128, 2, BP], bf16, tag="ht", name="ht")
                nc.scalar.activation(ht, pw[2][:, :, j], AF.Tanh)
                zt2 = ew_pool.tile([128, 2, BP], bf16, tag="z2", name="z2")
                nc.vector.tensor_mul(zt2, z_t, ht)
                nc.vector.tensor_add(h_out, zt2, hkeep)
                prev_parts[0] = (zt2, hkeep)
                if j == WSTEPS - 1:
                    win[w] = None
                    xet_w[w] = None

            # ---- software pipeline ----
            emit_gather(0)
            if n_windows > 1:
                emit_gather(1)
            for op in proj_ops(0):
                op()
            pq = proj_ops(1) if n_windows > 1 else []
            for w in range(n_windows):
                if w + 2 < n_windows:
                    emit_gather(w + 2)
                for j in range(WSTEPS):
                    emit_step(w * WSTEPS + j, pq)
                assert not pq, f"proj queue not drained for window {w + 1}"
                pq = proj_ops(w + 2) if w + 2 < n_windows else []

            h_fin = hbuf[n_steps % 2]

            # ---- FC: out[v, b] = sum_k Wf.T[k, v] h[k, b] (bias on host) ----
            with (
                tc.tile_pool(name="pfc", bufs=2, space="PSUM") as pfc_pool,
                tc.tile_pool(name="fcout", bufs=3) as fco_pool,
            ):
                ngroups = (FC_TILES + FC_GROUP - 1) // FC_GROUP
                for G in range(ngroups):
                    t0 = G * FC_GROUP
                    nt = min(FC_TILES, t0 + FC_GROUP) - t0
                    pf = pfc_pool.tile([128, FC_GROUP, BP], f32, tag="pf",
                                       name="pf")
                    for ti in range(nt):
                        nc.tensor.matmul(pf[:, ti], wf_sb[:, 0, t0 + ti],
                                         h_fin[:, 0], start=(ti == 0),
                                         stop=False, skip_group_check=True)
                        nc.tensor.matmul(pf[:, ti], wf_sb[:, 1, t0 + ti],
                                         h_fin[:, 1], start=False,
                                         stop=(ti == nt - 1),
                                         skip_group_check=True)
                    ot = fco_pool.tile([128, FC_GROUP, BP], f32, tag="ot",
                                       name="ot")
                    if G % 2 == 0:
                        nc.scalar.copy(ot[:, :nt], pf[:, :nt])
                    else:
                        nc.vector.tensor_scalar_add(ot[:, :nt], pf[:, :nt], 0.0)
                    nc.sync.dma_start(out=out_d[:, t0:t0 + nt],
                                      in_=ot[:, :nt])

    nc.compile()
    return nc


def prep_inputs(x, emb, Wz, bz, Uz, buz, Wr, br, Ur, bur, Wh, bh, Uh, buh,
                Wf, bf, n_windows=NW):
    """Host-side weight prep -> per-core input maps."""
    import ml_dtypes
    bf16 = ml_dtypes.bfloat16

    emb_bf = np.ascontiguousarray(emb, dtype=np.float32).astype(bf16)

    # proj lhsT pack: [128, k(2) x gate(3) x H]; element [p, k*768+g*256+h]
    # = Wg[h, k*128+p]
    wt_pack = np.zeros((128, 2 * 3 * H), dtype=np.float32)
    for k in range(2):
        for g, W in enumerate([Wz, Wr, Wh]):
            wt_pack[:, k * 3 * H + g * H:k * 3 * H + (g + 1) * H] = \
                np.asarray(W, np.float32).T[k * 128:(k + 1) * 128, :]
    bias_row = np.concatenate([
        np.asarray(bz, np.float32) + np.asarray(buz, np.float32),
        np.asarray(br, np.float32) + np.asarray(bur, np.float32),
        np.asarray(bh, np.float32) + np.asarray(buh, np.float32),
    ])[None, :]

    # rec lhsT pack: [128, g(3) x k(2) x H]; tile (g,k,m) = Ug.T[k-rows, m-cols]
    ut_pack = np.zeros((128, 3 * 2 * H), dtype=np.float32)
    for g, U in enumerate([Uz, Ur, Uh]):
        for k in range(2):
            ut_pack[:, g * 2 * H + k * H:g * 2 * H + (k + 1) * H] = \
                np.asarray(U, np.float32).T[k * 128:(k + 1) * 128, :]

    # FC weights: Wf.T [256, 32000] -> [128, k-half(2), tile(250), m(128)]
    wft = np.asarray(Wf, np.float32).T.reshape(2, 128, FC_TILES, 128)
    wft = np.ascontiguousarray(wft.transpose(1, 0, 2, 3))

    row0 = np.concatenate(
        [bias_row, np.ones((1, WTOK), np.float32)], axis=1).astype(bf16)

    f8 = ml_dtypes.float8_e4m3
    n_idx_cols = n_windows * (WTOK // 16)
    const_base = np.zeros((128, 2304 + n_idx_cols), dtype=np.int16)
    const_base[:, 0:1536] = wt_pack.astype(bf16).view(np.int16)
    const_base[:, 1536:2304] = np.ascontiguousarray(
        ut_pack.astype(f8)).view(np.int16)

    shared = {
        "emb_bf": emb_bf,
        "row0": row0,
        "wft": wft.astype(bf16),
    }

    n_steps = n_windows * WSTEPS
    x = np.asarray(x)
    in_maps = []
    for c in range(NCORES):
        xs = x[c * BP:(c + 1) * BP, :n_steps]      # [BP, n_steps]
        toks = np.transpose(xs, (1, 0)).reshape(n_windows, WTOK)
        idx = np.zeros((128, n_idx_cols), dtype=np.int16)
        for w in range(n_windows):
            blk = toks[w].reshape(WTOK // 16, 16).T  # [16, WTOK//16]
            # each of the 8 Q7 cores reads its own 16-partition group
            idx[:, w * (WTOK // 16):(w + 1) * (WTOK // 16)] = np.tile(blk, (8, 1))
        const2d = const_base.copy()
        const2d[:, 2304:] = idx
        in_maps.append({**shared, "const2d": const2d})
    return in_maps


_CACHED = {}


def kernel(**inputs):
    from concourse.bass_utils import run_bass_kernel_spmd

    if "nc" not in _CACHED:
        _CACHED["nc"] = build_kernel()
    nc = _CACHED["nc"]
    in_maps = prep_inputs(**inputs)
    res = run_bass_kernel_spmd(nc, in_maps, list(range(NCORES)))
    bf = np.asarray(inputs["bf"], np.float32)
    out = np.empty((B, O), np.float32)
    for c in range(NCORES):
        arr = res.results[c]["out"]                   # [128, 250, 32]
        vmaj = arr.transpose(1, 0, 2).reshape(O, BP)  # [vocab, batch]
        out[c * BP:(c + 1) * BP, :] = vmaj.T + bf[None, :]
    return out


if __name__ == "__main__":
    print("kernel module OK")

